# revision 30
# baseline (speedup 1.0000x reference)
"""MultiHeadAttention (pre-LN, residual) Trainium2 Bass kernel, 8 NeuronCores.

Problem: q,k,v [2, 2048, 1024], 16 heads x 64 dim, LN(q) -> QKV proj ->
softmax attention -> out proj -> +residual(q).

Sharding: core c owns tokens [512c, 512c+512) of the flattened [4096, 1024]
token axis (batch 0 = cores 0-3, batch 1 = cores 4-7).  All projections are
token-sharded (each core projects its 512 tokens for ALL heads).  The K / V
projections are then AllGathered *within each batch group of 4 cores*, so
every core holds its batch's full K^T / V and computes attention + output
projection for its own 512 query tokens.  No cross-core reduction is needed;
each core returns its 512 output rows.

Layout convention on device: "T layout" = features on partitions, tokens on
free axis.  PE matmuls contract over partitions, so:
  S^T tile [keys, q] = matmul(lhsT=K^T [dk, keys], rhs=Q^T [dk, q])
  O^T [dv, q]       += matmul(lhsT=V  [keys, dv],  rhs=exp(S^T) [keys, q])
  denom [1, q]      += matmul(lhsT=ones [keys, 1], rhs=exp(S^T) [keys, q])
Softmax is unnormalized exp (no max subtraction: S/tau is ~N(0,1), well
within fp32 exp range), normalized at the end by 1/denom broadcast via a
K=1 ones matmul.
"""

import numpy as np

N_CORES = 8
B, L, D = 2, 2048, 1024
H, DK, DV = 16, 64, 64
NT = B * L            # 4096 flattened tokens
TPC = NT // N_CORES   # 512 tokens per core
GROUP = 4             # cores per batch group
LB = L                # keys per batch (2048)
P = 128
NDT = D // P          # 8 d-tiles of 128
NMT = D // P          # 8 output-feature tiles
NTT = TPC // P        # 4 token tiles of 128 per core
NKT = LB // P         # 16 key tiles of 128 per batch
NHP = H // 2          # 8 head pairs
EPS = 1e-6
TAU_INV = 1.0 / float(np.sqrt(DK))

_CACHE = {}


def _np_reference(q, k, v, mask, w_q, w_k, w_v, w_o, ln_g, ln_b):
    """Pure-numpy fallback (only used if mask isn't all-ones)."""
    q64 = q.astype(np.float64)
    mu = q64.mean(-1, keepdims=True)
    var = q64.var(-1, keepdims=True)
    qn = (q64 - mu) / np.sqrt(var + EPS) * ln_g + ln_b
    Q = (qn @ w_q.T.astype(np.float64)).reshape(B, L, H, DK).transpose(0, 2, 1, 3)
    K = (k.astype(np.float64) @ w_k.T.astype(np.float64)).reshape(B, L, H, DK).transpose(0, 2, 1, 3)
    V = (v.astype(np.float64) @ w_v.T.astype(np.float64)).reshape(B, L, H, DV).transpose(0, 2, 1, 3)
    S = np.einsum("bhqd,bhkd->bhqk", Q / np.sqrt(DK), K)
    S = np.where(mask[None, None] == 0, -1e9, S)
    S = S - S.max(-1, keepdims=True)
    Pm = np.exp(S)
    Pm = Pm / Pm.sum(-1, keepdims=True)
    O = np.einsum("bhqk,bhkd->bhqd", Pm, V)
    O = O.transpose(0, 2, 1, 3).reshape(B, L, H * DV)
    out = O @ w_o.T.astype(np.float64) + q64
    return out.astype(np.float32)


def build_nc():
    import concourse.bass as bass
    import concourse.mybir as mybir
    import concourse.tile as tile
    from concourse import bacc
    from concourse.masks import make_identity

    f32 = mybir.dt.float32
    bf16 = mybir.dt.bfloat16

    nc = bacc.Bacc(num_devices=N_CORES)

    q_c = nc.declare_dram_parameter("q_c", [TPC, D], f32, isOutput=False)
    kT_c = nc.declare_dram_parameter("kT_c", [D, TPC], bf16, isOutput=False)
    vT_c = nc.declare_dram_parameter("vT_c", [D, TPC], bf16, isOutput=False)
    wgqT = nc.declare_dram_parameter("wgqT", [D, D], bf16, isOutput=False)
    wkT = nc.declare_dram_parameter("wkT", [D, D], bf16, isOutput=False)
    wvT = nc.declare_dram_parameter("wvT", [D, D], bf16, isOutput=False)
    woT = nc.declare_dram_parameter("woT", [D, D], bf16, isOutput=False)
    cq = nc.declare_dram_parameter("cq", [D], f32, isOutput=False)
    out_c = nc.declare_dram_parameter("out_c", [TPC, D], f32, isOutput=True)

    RG = [[0, 1, 2, 3], [4, 5, 6, 7]]

    with tile.TileContext(nc) as tc:
        with tc.tile_pool(name="dram", bufs=1, space="DRAM") as dram:
            kag_in = dram.tile([D, TPC], bf16)              # K^T shard (all heads, my tokens)
            vag_in = dram.tile([TPC, D], bf16)              # V natural shard
            kag_out = dram.tile([GROUP, D, TPC], bf16)
            vag_out = dram.tile([LB, D], bf16)

            with tc.tile_pool(name="singles", bufs=1) as singles:
                ident = singles.tile([P, P], f32)
                make_identity(nc, ident)
                ones_sb = singles.tile([P, P], bf16)
                nc.vector.memset(ones_sb, 1.0)
                ones_f32 = singles.tile([P, DK], f32)
                nc.vector.memset(ones_f32, 1.0)
                eps_sb = singles.tile([P, 1], f32)
                nc.vector.memset(eps_sb, EPS)
                cq_sb = singles.tile([P, NMT], f32)
                nc.sync.dma_start(out=cq_sb, in_=cq.rearrange("(mt p) -> p mt", p=P))

                # ---- persistent sbuf (live across phases) ----
                with tc.tile_pool(name="persist", bufs=1) as persist:
                    q_sb = persist.tile([P, NTT, D], f32)      # residual + LN input
                    qT_sb = persist.tile([P, NMT, TPC], bf16)   # Q^T (all heads, my tokens)
                    aO_sb = persist.tile([P, NHP, TPC], bf16)   # attn out^T (dv-concat, my tokens)

                    nc.sync.dma_start(
                        out=q_sb, in_=q_c.rearrange("(tt p) d -> p tt d", p=P)
                    )

                    # ================= Phase 1: K / V projections + AllGather ====
                    with tc.tile_pool(name="p1", bufs=1) as p1, \
                         tc.tile_pool(name="p1psum", bufs=2, space="PSUM") as p1psum:
                        wk_sb = p1.tile([P, NDT, D], bf16)
                        ktc_sb = p1.tile([P, NDT, TPC], bf16)
                        wkr = wkT.rearrange("(dt p) m -> p dt m", p=P)
                        ktr = kT_c.rearrange("(dt p) t -> p dt t", p=P)
                        for dt in range(NDT):
                            nc.sync.dma_start(out=wk_sb[:, dt, :], in_=wkr[:, dt, :])
                            nc.sync.dma_start(out=ktc_sb[:, dt, :], in_=ktr[:, dt, :])
                        kc_sb = p1.tile([P, NMT, TPC], bf16)
                        for mt in range(NMT):
                            ps = p1psum.tile([P, TPC], f32, tag="ps")
                            for dt in range(NDT):
                                nc.tensor.matmul(
                                    ps,
                                    wk_sb[:, dt, mt * P:(mt + 1) * P],
                                    ktc_sb[:, dt, :],
                                    start=(dt == 0),
                                    stop=(dt == NDT - 1),
                                )
                            nc.vector.tensor_copy(kc_sb[:, mt, :], ps)
                        nc.sync.dma_start(
                            out=kag_in.rearrange("(mt p) t -> p mt t", p=P),
                            in_=kc_sb,
                        )
                        nc.gpsimd.collective_compute(
                            "AllGather",
                            mybir.AluOpType.bypass,
                            replica_groups=RG,
                            ins=[kag_in[:, :].opt()],
                            outs=[kag_out[:, :, :].opt()],
                        )

                        wv_sb = p1.tile([P, NDT, D], bf16)
                        vtc_sb = p1.tile([P, NDT, TPC], bf16)
                        wvr = wvT.rearrange("(dt p) m -> p dt m", p=P)
                        vtr = vT_c.rearrange("(dt p) t -> p dt t", p=P)
                        for dt in range(NDT):
                            nc.sync.dma_start(out=wv_sb[:, dt, :], in_=wvr[:, dt, :])
                            nc.sync.dma_start(out=vtc_sb[:, dt, :], in_=vtr[:, dt, :])
                        vn_sb = p1.tile([P, NTT, D], bf16)
                        for tt in range(NTT):
                            for mc in range(2):  # dv-concat in two 512 chunks
                                ps = p1psum.tile([P, TPC], f32, tag="ps")
                                for dt in range(NDT):
                                    nc.tensor.matmul(
                                        ps,
                                        vtc_sb[:, dt, tt * P:(tt + 1) * P],
                                        wv_sb[:, dt, mc * 512:(mc + 1) * 512],
                                        start=(dt == 0),
                                        stop=(dt == NDT - 1),
                                    )
                                nc.vector.tensor_copy(
                                    vn_sb[:, tt, mc * 512:(mc + 1) * 512], ps
                                )
                        nc.sync.dma_start(
                            out=vag_in.rearrange("(tt p) d -> p tt d", p=P),
                            in_=vn_sb,
                        )
                        nc.gpsimd.collective_compute(
                            "AllGather",
                            mybir.AluOpType.bypass,
                            replica_groups=RG,
                            ins=[vag_in[:, :].opt()],
                            outs=[vag_out[:, :].opt()],
                        )

                    # ================= Phase 2: LayerNorm + Q projection ==========
                    with tc.tile_pool(name="p2", bufs=1) as p2, \
                         tc.tile_pool(name="p2w", bufs=1) as p2w, \
                         tc.tile_pool(name="p2s", bufs=4) as p2s, \
                         tc.tile_pool(name="p2psum", bufs=2, space="PSUM") as p2psum, \
                         tc.tile_pool(name="tpsum", bufs=2, space="PSUM") as tpsum:
                        qn_sb = p2.tile([P, NTT, D], f32)
                        for tt in range(NTT):
                            stats = p2s.tile([P, 2, 6], f32)
                            for sg in range(2):
                                nc.vector.bn_stats(
                                    out=stats[:, sg, :],
                                    in_=q_sb[:, tt, sg * 512:(sg + 1) * 512],
                                )
                            mv = p2s.tile([P, 2], f32)
                            nc.vector.bn_aggr(out=mv, in_=stats)
                            rstd = p2s.tile([P, 1], f32)
                            nc.scalar.activation(
                                out=rstd,
                                in_=mv[:, 1:2],
                                func=mybir.ActivationFunctionType.Sqrt,
                                bias=eps_sb,
                                scale=1.0,
                            )
                            nc.vector.reciprocal(out=rstd, in_=rstd)
                            nc.vector.tensor_scalar(
                                out=qn_sb[:, tt, :],
                                in0=q_sb[:, tt, :],
                                scalar1=mv[:, 0:1],
                                scalar2=rstd,
                                op0=mybir.AluOpType.subtract,
                                op1=mybir.AluOpType.mult,
                            )

                        # transpose qn -> qn^T [d on partitions, tokens free]
                        qnT_sb = p2.tile([P, NDT, TPC], bf16)
                        for tt in range(NTT):
                            for dt in range(NDT):
                                tp = tpsum.tile([P, P], f32, tag="tp")
                                nc.tensor.transpose(
                                    tp, qn_sb[:, tt, dt * P:(dt + 1) * P], ident
                                )
                                nc.vector.tensor_copy(
                                    qnT_sb[:, dt, tt * P:(tt + 1) * P], tp
                                )

                        wq_sb = p2w.tile([P, NDT, D], bf16)
                        nc.sync.dma_start(
                            out=wq_sb, in_=wgqT.rearrange("(dt p) m -> p dt m", p=P)
                        )
                        for mt in range(NMT):
                            ps = p2psum.tile([P, TPC], f32, tag="qps")
                            for dt in range(NDT):
                                nc.tensor.matmul(
                                    ps,
                                    wq_sb[:, dt, mt * P:(mt + 1) * P],
                                    qnT_sb[:, dt, :],
                                    start=(dt == 0),
                                    stop=(dt == NDT - 1),
                                )
                            # PSUM->SBUF + per-row bias (w_q @ ln_b)
                            nc.scalar.activation(
                                out=qT_sb[:, mt, :],
                                in_=ps,
                                func=mybir.ActivationFunctionType.Identity,
                                bias=cq_sb[:, mt:mt + 1],
                                scale=1.0,
                            )

                    # ================= Phase 3: attention =========================
                    with tc.tile_pool(name="kv", bufs=1) as kvp, \
                         tc.tile_pool(name="es", bufs=1) as es, \
                         tc.tile_pool(name="rp", bufs=2) as rp, \
                         tc.tile_pool(name="spsum", bufs=3, space="PSUM") as spsum, \
                         tc.tile_pool(name="opsum", bufs=1, space="PSUM") as opsum:
                        # Zero-padded full-array stationary operands and a
                        # 2-deep software pipeline over head pairs: S+exp for
                        # hp run 2 iterations ahead of the O matmuls (exp
                        # tiles buffered in SBUF), so ScalarE fills the
                        # AllGather-V wait and stays saturated after.
                        ksb_bufs = []
                        vsb_bufs = []
                        est_bufs = []
                        for i in range(2):
                            kb = kvp.tile([P, NKT, 2, P], bf16, name=f"ksb{i}")
                            nc.vector.memset(kb[DK:P, :, 0, :], 0.0)
                            nc.vector.memset(kb[0:DK, :, 1, :], 0.0)
                            vb = kvp.tile([P, NKT, 2, P], bf16, name=f"vsb{i}")
                            for h in range(2):
                                nc.vector.memset(vb[:, :, h, DK:DK + 1], 1.0)
                                nc.vector.memset(vb[:, :, h, DK + 1:P], 0.0)
                            eb = es.tile([P, NKT, 2, TPC], bf16, name=f"est{i}")
                            ksb_bufs.append(kb)
                            vsb_bufs.append(vb)
                            est_bufs.append(eb)

                        def emit_loads(hp):
                            ksb = ksb_bufs[hp % 2]
                            vsb = vsb_bufs[hp % 2]
                            for h in range(2):
                                for r in range(GROUP):
                                    nc.sync.dma_start(
                                        out=ksb[
                                            h * DK:(h + 1) * DK,
                                            r * NTT:(r + 1) * NTT, h, :,
                                        ],
                                        in_=kag_out[
                                            r, hp * P + h * DK:hp * P + (h + 1) * DK, :
                                        ].rearrange("p (tc c) -> p tc c", c=P),
                                    )
                                nc.sync.dma_start(
                                    out=vsb[:, :, h, 0:DK],
                                    in_=vag_out[
                                        :, hp * P + h * DK:hp * P + (h + 1) * DK
                                    ].rearrange("(t p) c -> p t c", p=P),
                                )

                        def emit_s_exp(hp):
                            ksb = ksb_bufs[hp % 2]
                            est = est_bufs[hp % 2]
                            for ktp in range(NKT // 2):
                                sAB = [
                                    spsum.tile([P, 2, TPC], f32, tag="s", name=f"sA_{hp}_{ktp}"),
                                    spsum.tile([P, 2, TPC], f32, tag="s", name=f"sB_{hp}_{ktp}"),
                                ]
                                for half in range(2):
                                    kt = 2 * ktp + half
                                    for h in range(2):
                                        nc.tensor.matmul(
                                            sAB[h][:, half, :],
                                            ksb[:, kt, h, :],
                                            qT_sb[:, hp, :],
                                            start=True,
                                            stop=True,
                                        )
                                for h in range(2):
                                    nc.scalar.activation(
                                        out=est[:, 2 * ktp, h, :],
                                        in_=sAB[h][:, 0, :],
                                        func=mybir.ActivationFunctionType.Exp,
                                        scale=TAU_INV,
                                    )
                                    nc.scalar.activation(
                                        out=est[:, 2 * ktp + 1, h, :],
                                        in_=sAB[h][:, 1, :],
                                        func=mybir.ActivationFunctionType.Exp,
                                        scale=TAU_INV,
                                    )

                        def emit_o(hp):
                            vsb = vsb_bufs[hp % 2]
                            est = est_bufs[hp % 2]
                            oAB = [
                                opsum.tile([P, TPC], f32, tag="oA", name=f"oA_{hp}"),
                                opsum.tile([P, TPC], f32, tag="oB", name=f"oB_{hp}"),
                            ]
                            for kt in range(NKT):
                                for h in range(2):
                                    nc.tensor.matmul(
                                        oAB[h],
                                        vsb[:, kt, h, :],
                                        est[:, kt, h, :],
                                        start=(kt == 0),
                                        stop=(kt == NKT - 1),
                                    )
                            return oAB

                        def emit_norm(hp, oAB):
                            rsb = rp.tile([P, 2, TPC], f32, tag="r", name=f"rsb{hp}")
                            for h in range(2):
                                nc.vector.reciprocal(
                                    out=rsb[0:1, h, :], in_=oAB[h][DK:DK + 1, :]
                                )
                            rbc = spsum.tile([P, TPC], f32, tag="s", name=f"rbc{hp}")
                            for h in range(2):
                                nc.tensor.matmul(
                                    rbc[DK * h:DK * (h + 1), :],
                                    ones_f32[0:1, :],
                                    rsb[0:1, h, :],
                                    start=True,
                                    stop=True,
                                    tile_position=(0, DK * h),
                                )
                            rbc_sb = rp.tile([P, TPC], f32, tag="rb", name=f"rbc_sb{hp}")
                            nc.vector.tensor_copy(rbc_sb, rbc)
                            for h in range(2):
                                nc.vector.tensor_mul(
                                    aO_sb[DK * h:DK * (h + 1), hp, :],
                                    oAB[h][0:DK, :],
                                    rbc_sb[DK * h:DK * (h + 1), :],
                                )

                        emit_loads(0)
                        emit_s_exp(0)
                        emit_loads(1)
                        emit_s_exp(1)
                        for hp in range(NHP):
                            oAB = emit_o(hp)
                            if hp + 2 < NHP:
                                emit_loads(hp + 2)
                                emit_s_exp(hp + 2)
                            emit_norm(hp, oAB)

                    # ================= Phase 4: out projection + residual =========
                    with tc.tile_pool(name="p4", bufs=1) as p4, \
                         tc.tile_pool(name="p4o", bufs=2) as p4o, \
                         tc.tile_pool(name="p4psum", bufs=2, space="PSUM") as p4psum:
                        wo_sb = p4.tile([P, NDT, D], bf16)
                        nc.sync.dma_start(
                            out=wo_sb, in_=woT.rearrange("(dt p) m -> p dt m", p=P)
                        )
                        for tt in range(NTT):
                            ob = p4o.tile([P, D], f32, tag="ob")
                            for mc in range(2):
                                ps = p4psum.tile([P, TPC], f32, tag="ops")
                                for dt in range(NDT):
                                    nc.tensor.matmul(
                                        ps,
                                        aO_sb[:, dt, tt * P:(tt + 1) * P],
                                        wo_sb[:, dt, mc * 512:(mc + 1) * 512],
                                        start=(dt == 0),
                                        stop=(dt == NDT - 1),
                                    )
                                nc.vector.tensor_add(
                                    ob[:, mc * 512:(mc + 1) * 512],
                                    ps,
                                    q_sb[:, tt, mc * 512:(mc + 1) * 512],
                                )
                            nc.sync.dma_start(
                                out=out_c[tt * P:(tt + 1) * P, :], in_=ob
                            )

    nc.compile()
    return nc


def _get_nc():
    if "nc" not in _CACHE:
        _CACHE["nc"] = build_nc()
    return _CACHE["nc"]


def make_in_maps(q, k, v, w_q, w_k, w_v, w_o, ln_g, ln_b):
    import ml_dtypes

    bf = ml_dtypes.bfloat16
    q2 = np.ascontiguousarray(q.reshape(NT, D), dtype=np.float32)
    kT = np.ascontiguousarray(k.reshape(NT, D).T.astype(bf))
    vT = np.ascontiguousarray(v.reshape(NT, D).T.astype(bf))
    wgqT = np.ascontiguousarray((w_q * ln_g[None, :]).T.astype(bf))
    wkT = np.ascontiguousarray(w_k.T.astype(bf))
    wvT = np.ascontiguousarray(w_v.T.astype(bf))
    woT = np.ascontiguousarray(w_o.T.astype(bf))
    cq = np.ascontiguousarray(w_q @ ln_b, dtype=np.float32)
    in_maps = []
    for c in range(N_CORES):
        sl = slice(c * TPC, (c + 1) * TPC)
        in_maps.append(
            {
                "q_c": q2[sl],
                "kT_c": np.ascontiguousarray(kT[:, sl]),
                "vT_c": np.ascontiguousarray(vT[:, sl]),
                "wgqT": wgqT,
                "wkT": wkT,
                "wvT": wvT,
                "woT": woT,
                "cq": cq,
            }
        )
    return in_maps


def run(inputs, trace=False, tmpdir=None):
    """Run the device kernel.  Returns (out [B, L, D], BassKernelResults)."""
    from concourse.bass_utils import run_bass_kernel_spmd

    nc = _get_nc()
    in_maps = make_in_maps(
        inputs["q"], inputs["k"], inputs["v"], inputs["w_q"], inputs["w_k"],
        inputs["w_v"], inputs["w_o"], inputs["ln_g"], inputs["ln_b"],
    )
    res = run_bass_kernel_spmd(
        nc, in_maps, list(range(N_CORES)), trace=trace, tmpdir=tmpdir
    )
    rows = np.concatenate([res.results[c]["out_c"] for c in range(N_CORES)], axis=0)
    return rows.reshape(B, L, D), res


def kernel(q, k, v, mask, w_q, w_k, w_v, w_o, ln_g, ln_b):
    q = np.asarray(q, dtype=np.float32)
    k = np.asarray(k, dtype=np.float32)
    v = np.asarray(v, dtype=np.float32)
    mask = np.asarray(mask)
    w_q = np.asarray(w_q, dtype=np.float32)
    w_k = np.asarray(w_k, dtype=np.float32)
    w_v = np.asarray(w_v, dtype=np.float32)
    w_o = np.asarray(w_o, dtype=np.float32)
    ln_g = np.asarray(ln_g, dtype=np.float32)
    ln_b = np.asarray(ln_b, dtype=np.float32)
    if not np.all(mask == 1):
        return _np_reference(q, k, v, mask, w_q, w_k, w_v, w_o, ln_g, ln_b)
    out, _ = run(
        {"q": q, "k": k, "v": v, "w_q": w_q, "w_k": w_k, "w_v": w_v,
         "w_o": w_o, "ln_g": ln_g, "ln_b": ln_b},
        trace=False,
    )
    return out


# revision 31
# speedup vs baseline: 1.0854x; 1.0854x over previous
"""MultiHeadAttention (pre-LN, residual) Trainium2 Bass kernel, 8 NeuronCores.

Problem: q,k,v [2, 2048, 1024], 16 heads x 64 dim, LN(q) -> QKV proj ->
softmax attention -> out proj -> +residual(q).

Sharding: core c owns tokens [512c, 512c+512) of the flattened [4096, 1024]
token axis (batch 0 = cores 0-3, batch 1 = cores 4-7).  All projections are
token-sharded (each core projects its 512 tokens for ALL heads).  The K / V
projections are then AllGathered *within each batch group of 4 cores*, so
every core holds its batch's full K^T / V and computes attention + output
projection for its own 512 query tokens.  No cross-core reduction is needed;
each core returns its 512 output rows.

Layout convention on device: "T layout" = features on partitions, tokens on
free axis.  PE matmuls contract over partitions, so:
  S^T tile [keys, q] = matmul(lhsT=K^T [dk, keys], rhs=Q^T [dk, q])
  O^T [dv, q]       += matmul(lhsT=V  [keys, dv],  rhs=exp(S^T) [keys, q])
  denom [1, q]      += matmul(lhsT=ones [keys, 1], rhs=exp(S^T) [keys, q])
Softmax is unnormalized exp (no max subtraction: S/tau is ~N(0,1), well
within fp32 exp range), normalized at the end by 1/denom broadcast via a
K=1 ones matmul.
"""

import numpy as np

N_CORES = 8
B, L, D = 2, 2048, 1024
H, DK, DV = 16, 64, 64
NT = B * L            # 4096 flattened tokens
TPC = NT // N_CORES   # 512 tokens per core
GROUP = 4             # cores per batch group
LB = L                # keys per batch (2048)
P = 128
NDT = D // P          # 8 d-tiles of 128
NMT = D // P          # 8 output-feature tiles
NTT = TPC // P        # 4 token tiles of 128 per core
NKT = LB // P         # 16 key tiles of 128 per batch
NHP = H // 2          # 8 head pairs
EPS = 1e-6
TAU_INV = 1.0 / float(np.sqrt(DK))

_CACHE = {}


def _np_reference(q, k, v, mask, w_q, w_k, w_v, w_o, ln_g, ln_b):
    """Pure-numpy fallback (only used if mask isn't all-ones)."""
    q64 = q.astype(np.float64)
    mu = q64.mean(-1, keepdims=True)
    var = q64.var(-1, keepdims=True)
    qn = (q64 - mu) / np.sqrt(var + EPS) * ln_g + ln_b
    Q = (qn @ w_q.T.astype(np.float64)).reshape(B, L, H, DK).transpose(0, 2, 1, 3)
    K = (k.astype(np.float64) @ w_k.T.astype(np.float64)).reshape(B, L, H, DK).transpose(0, 2, 1, 3)
    V = (v.astype(np.float64) @ w_v.T.astype(np.float64)).reshape(B, L, H, DV).transpose(0, 2, 1, 3)
    S = np.einsum("bhqd,bhkd->bhqk", Q / np.sqrt(DK), K)
    S = np.where(mask[None, None] == 0, -1e9, S)
    S = S - S.max(-1, keepdims=True)
    Pm = np.exp(S)
    Pm = Pm / Pm.sum(-1, keepdims=True)
    O = np.einsum("bhqk,bhkd->bhqd", Pm, V)
    O = O.transpose(0, 2, 1, 3).reshape(B, L, H * DV)
    out = O @ w_o.T.astype(np.float64) + q64
    return out.astype(np.float32)


def build_nc():
    import concourse.bass as bass
    import concourse.mybir as mybir
    import concourse.tile as tile
    from concourse import bacc
    from concourse.masks import make_identity

    f32 = mybir.dt.float32
    bf16 = mybir.dt.bfloat16

    nc = bacc.Bacc(num_devices=N_CORES)

    q_c = nc.declare_dram_parameter("q_c", [TPC, D], f32, isOutput=False)
    kT_c = nc.declare_dram_parameter("kT_c", [D, TPC], bf16, isOutput=False)
    vT_c = nc.declare_dram_parameter("vT_c", [D, TPC], bf16, isOutput=False)
    wgqT = nc.declare_dram_parameter("wgqT", [D, D], bf16, isOutput=False)
    wkT = nc.declare_dram_parameter("wkT", [D, D], bf16, isOutput=False)
    wvT = nc.declare_dram_parameter("wvT", [D, D], bf16, isOutput=False)
    woT = nc.declare_dram_parameter("woT", [D, D], bf16, isOutput=False)
    cq = nc.declare_dram_parameter("cq", [D], f32, isOutput=False)
    out_c = nc.declare_dram_parameter("out_c", [TPC, D], f32, isOutput=True)

    RG = [[0, 1, 2, 3], [4, 5, 6, 7]]

    with tile.TileContext(nc) as tc:
        with tc.tile_pool(name="dram", bufs=1, space="DRAM") as dram:
            kag_in = dram.tile([D, TPC], bf16)              # K^T shard (all heads, my tokens)
            vag_in = dram.tile([TPC, D], bf16)              # V natural shard
            kag_out = dram.tile([GROUP, D, TPC], bf16)
            vag_out = dram.tile([LB, D], bf16)

            with tc.tile_pool(name="singles", bufs=1) as singles:
                ident = singles.tile([P, P], f32)
                make_identity(nc, ident)
                ones_sb = singles.tile([P, P], bf16)
                nc.vector.memset(ones_sb, 1.0)
                ones_f32 = singles.tile([P, DK], f32)
                nc.vector.memset(ones_f32, 1.0)
                eps_sb = singles.tile([P, 1], f32)
                nc.vector.memset(eps_sb, EPS)
                cq_sb = singles.tile([P, NMT], f32)
                nc.sync.dma_start(out=cq_sb, in_=cq.rearrange("(mt p) -> p mt", p=P))

                # ---- persistent sbuf (live across phases) ----
                with tc.tile_pool(name="persist", bufs=1) as persist:
                    q_sb = persist.tile([P, NTT, D], f32)      # residual + LN input
                    qT_sb = persist.tile([P, NMT, TPC], bf16)   # Q^T (all heads, my tokens)
                    aO_sb = persist.tile([P, NHP, TPC], bf16)   # attn out^T (dv-concat, my tokens)

                    nc.sync.dma_start(
                        out=q_sb, in_=q_c.rearrange("(tt p) d -> p tt d", p=P)
                    )

                    # ================= Phase 1: K / V projections + AllGather ====
                    with tc.tile_pool(name="p1", bufs=1) as p1, \
                         tc.tile_pool(name="p1psum", bufs=2, space="PSUM") as p1psum:
                        wk_sb = p1.tile([P, NDT, D], bf16)
                        ktc_sb = p1.tile([P, NDT, TPC], bf16)
                        wkr = wkT.rearrange("(dt p) m -> p dt m", p=P)
                        ktr = kT_c.rearrange("(dt p) t -> p dt t", p=P)
                        for dt in range(NDT):
                            nc.sync.dma_start(out=wk_sb[:, dt, :], in_=wkr[:, dt, :])
                            nc.sync.dma_start(out=ktc_sb[:, dt, :], in_=ktr[:, dt, :])
                        kc_sb = p1.tile([P, NMT, TPC], bf16)
                        for mt in range(NMT):
                            ps = p1psum.tile([P, TPC], f32, tag="ps")
                            for dt in range(NDT):
                                nc.tensor.matmul(
                                    ps,
                                    wk_sb[:, dt, mt * P:(mt + 1) * P],
                                    ktc_sb[:, dt, :],
                                    start=(dt == 0),
                                    stop=(dt == NDT - 1),
                                )
                            nc.vector.tensor_copy(kc_sb[:, mt, :], ps)
                        nc.sync.dma_start(
                            out=kag_in.rearrange("(mt p) t -> p mt t", p=P),
                            in_=kc_sb,
                        )
                        nc.gpsimd.collective_compute(
                            "AllGather",
                            mybir.AluOpType.bypass,
                            replica_groups=RG,
                            ins=[kag_in[:, :].opt()],
                            outs=[kag_out[:, :, :].opt()],
                        )

                        wv_sb = p1.tile([P, NDT, D], bf16)
                        vtc_sb = p1.tile([P, NDT, TPC], bf16)
                        wvr = wvT.rearrange("(dt p) m -> p dt m", p=P)
                        vtr = vT_c.rearrange("(dt p) t -> p dt t", p=P)
                        for dt in range(NDT):
                            nc.sync.dma_start(out=wv_sb[:, dt, :], in_=wvr[:, dt, :])
                            nc.sync.dma_start(out=vtc_sb[:, dt, :], in_=vtr[:, dt, :])
                        vn_sb = p1.tile([P, NTT, D], bf16)
                        for tt in range(NTT):
                            for mc in range(2):  # dv-concat in two 512 chunks
                                ps = p1psum.tile([P, TPC], f32, tag="ps")
                                for dt in range(NDT):
                                    nc.tensor.matmul(
                                        ps,
                                        vtc_sb[:, dt, tt * P:(tt + 1) * P],
                                        wv_sb[:, dt, mc * 512:(mc + 1) * 512],
                                        start=(dt == 0),
                                        stop=(dt == NDT - 1),
                                    )
                                nc.vector.tensor_copy(
                                    vn_sb[:, tt, mc * 512:(mc + 1) * 512], ps
                                )
                        nc.sync.dma_start(
                            out=vag_in.rearrange("(tt p) d -> p tt d", p=P),
                            in_=vn_sb,
                        )
                        nc.gpsimd.collective_compute(
                            "AllGather",
                            mybir.AluOpType.bypass,
                            replica_groups=RG,
                            ins=[vag_in[:, :].opt()],
                            outs=[vag_out[:, :].opt()],
                        )

                    # ================= Phase 2: LayerNorm + Q projection ==========
                    with tc.tile_pool(name="p2", bufs=1) as p2, \
                         tc.tile_pool(name="p2w", bufs=1) as p2w, \
                         tc.tile_pool(name="p2s", bufs=4) as p2s, \
                         tc.tile_pool(name="p2psum", bufs=2, space="PSUM") as p2psum, \
                         tc.tile_pool(name="tpsum", bufs=2, space="PSUM") as tpsum:
                        qn_sb = p2.tile([P, NTT, D], f32)
                        for tt in range(NTT):
                            stats = p2s.tile([P, 2, 6], f32)
                            for sg in range(2):
                                nc.vector.bn_stats(
                                    out=stats[:, sg, :],
                                    in_=q_sb[:, tt, sg * 512:(sg + 1) * 512],
                                )
                            mv = p2s.tile([P, 2], f32)
                            nc.vector.bn_aggr(out=mv, in_=stats)
                            rstd = p2s.tile([P, 1], f32)
                            nc.scalar.activation(
                                out=rstd,
                                in_=mv[:, 1:2],
                                func=mybir.ActivationFunctionType.Sqrt,
                                bias=eps_sb,
                                scale=1.0,
                            )
                            nc.vector.reciprocal(out=rstd, in_=rstd)
                            nc.vector.tensor_scalar(
                                out=qn_sb[:, tt, :],
                                in0=q_sb[:, tt, :],
                                scalar1=mv[:, 0:1],
                                scalar2=rstd,
                                op0=mybir.AluOpType.subtract,
                                op1=mybir.AluOpType.mult,
                            )

                        # transpose qn -> qn^T [d on partitions, tokens free]
                        qnT_sb = p2.tile([P, NDT, TPC], bf16)
                        for tt in range(NTT):
                            for dt in range(NDT):
                                tp = tpsum.tile([P, P], f32, tag="tp")
                                nc.tensor.transpose(
                                    tp, qn_sb[:, tt, dt * P:(dt + 1) * P], ident
                                )
                                nc.vector.tensor_copy(
                                    qnT_sb[:, dt, tt * P:(tt + 1) * P], tp
                                )

                        wq_sb = p2w.tile([P, NDT, D], bf16)
                        nc.sync.dma_start(
                            out=wq_sb, in_=wgqT.rearrange("(dt p) m -> p dt m", p=P)
                        )
                        for mt in range(NMT):
                            ps = p2psum.tile([P, TPC], f32, tag="qps")
                            for dt in range(NDT):
                                nc.tensor.matmul(
                                    ps,
                                    wq_sb[:, dt, mt * P:(mt + 1) * P],
                                    qnT_sb[:, dt, :],
                                    start=(dt == 0),
                                    stop=(dt == NDT - 1),
                                )
                            # PSUM->SBUF + per-row bias (w_q @ ln_b)
                            nc.scalar.activation(
                                out=qT_sb[:, mt, :],
                                in_=ps,
                                func=mybir.ActivationFunctionType.Identity,
                                bias=cq_sb[:, mt:mt + 1],
                                scale=1.0,
                            )

                    # ================= Phase 3: attention =========================
                    with tc.tile_pool(name="kv", bufs=1) as kvp, \
                         tc.tile_pool(name="es", bufs=1) as es, \
                         tc.tile_pool(name="rp", bufs=2) as rp, \
                         tc.tile_pool(name="spsum", bufs=3, space="PSUM") as spsum, \
                         tc.tile_pool(name="opsum", bufs=1, space="PSUM") as opsum:
                        # Zero-padded full-array stationary operands and a
                        # 2-deep software pipeline over head pairs: S+exp for
                        # hp run 2 iterations ahead of the O matmuls (exp
                        # tiles buffered in SBUF), so ScalarE fills the
                        # AllGather-V wait and stays saturated after.
                        ksb_bufs = []
                        vsb_bufs = []
                        est_bufs = []
                        for i in range(2):
                            kb = kvp.tile([P, NKT, 2, P], bf16, name=f"ksb{i}")
                            nc.vector.memset(kb[DK:P, :, 0, :], 0.0)
                            nc.vector.memset(kb[0:DK, :, 1, :], 0.0)
                            vb = kvp.tile([P, NKT, 2, P], bf16, name=f"vsb{i}")
                            for h in range(2):
                                nc.vector.memset(vb[:, :, h, DK:DK + 1], 1.0)
                                nc.vector.memset(vb[:, :, h, DK + 1:P], 0.0)
                            eb = es.tile([P, NKT, 2, TPC], bf16, name=f"est{i}")
                            ksb_bufs.append(kb)
                            vsb_bufs.append(vb)
                            est_bufs.append(eb)

                        def emit_loads(hp):
                            ksb = ksb_bufs[hp % 2]
                            vsb = vsb_bufs[hp % 2]
                            for h in range(2):
                                for r in range(GROUP):
                                    nc.sync.dma_start(
                                        out=ksb[
                                            h * DK:(h + 1) * DK,
                                            r * NTT:(r + 1) * NTT, h, :,
                                        ],
                                        in_=kag_out[
                                            r, hp * P + h * DK:hp * P + (h + 1) * DK, :
                                        ].rearrange("p (tc c) -> p tc c", c=P),
                                    )
                                nc.sync.dma_start(
                                    out=vsb[:, :, h, 0:DK],
                                    in_=vag_out[
                                        :, hp * P + h * DK:hp * P + (h + 1) * DK
                                    ].rearrange("(t p) c -> p t c", p=P),
                                )

                        def emit_s_exp(hp):
                            ksb = ksb_bufs[hp % 2]
                            est = est_bufs[hp % 2]
                            for ktp in range(NKT // 2):
                                sAB = [
                                    spsum.tile([P, 2, TPC], f32, tag="s", name=f"sA_{hp}_{ktp}"),
                                    spsum.tile([P, 2, TPC], f32, tag="s", name=f"sB_{hp}_{ktp}"),
                                ]
                                for half in range(2):
                                    kt = 2 * ktp + half
                                    for h in range(2):
                                        nc.tensor.matmul(
                                            sAB[h][:, half, :],
                                            ksb[:, kt, h, :],
                                            qT_sb[:, hp, :],
                                            start=True,
                                            stop=True,
                                        )
                                for h in range(2):
                                    nc.scalar.activation(
                                        out=est[:, 2 * ktp:2 * ktp + 2, h, :],
                                        in_=sAB[h],
                                        func=mybir.ActivationFunctionType.Exp,
                                        scale=TAU_INV,
                                    )

                        def emit_o(hp):
                            vsb = vsb_bufs[hp % 2]
                            est = est_bufs[hp % 2]
                            oAB = [
                                opsum.tile([P, TPC], f32, tag="oA", name=f"oA_{hp}"),
                                opsum.tile([P, TPC], f32, tag="oB", name=f"oB_{hp}"),
                            ]
                            for kt in range(NKT):
                                for h in range(2):
                                    nc.tensor.matmul(
                                        oAB[h],
                                        vsb[:, kt, h, :],
                                        est[:, kt, h, :],
                                        start=(kt == 0),
                                        stop=(kt == NKT - 1),
                                    )
                            return oAB

                        def emit_norm(hp, oAB):
                            rsb = rp.tile([P, 2, TPC], f32, tag="r", name=f"rsb{hp}")
                            for h in range(2):
                                nc.vector.reciprocal(
                                    out=rsb[0:1, h, :], in_=oAB[h][DK:DK + 1, :]
                                )
                            rbc = spsum.tile([P, TPC], f32, tag="s", name=f"rbc{hp}")
                            for h in range(2):
                                nc.tensor.matmul(
                                    rbc[DK * h:DK * (h + 1), :],
                                    ones_f32[0:1, :],
                                    rsb[0:1, h, :],
                                    start=True,
                                    stop=True,
                                    tile_position=(0, DK * h),
                                )
                            rbc_sb = rp.tile([P, TPC], f32, tag="rb", name=f"rbc_sb{hp}")
                            nc.vector.tensor_copy(rbc_sb, rbc)
                            for h in range(2):
                                nc.vector.tensor_mul(
                                    aO_sb[DK * h:DK * (h + 1), hp, :],
                                    oAB[h][0:DK, :],
                                    rbc_sb[DK * h:DK * (h + 1), :],
                                )

                        emit_loads(0)
                        emit_s_exp(0)
                        emit_loads(1)
                        emit_s_exp(1)
                        for hp in range(NHP):
                            oAB = emit_o(hp)
                            if hp + 2 < NHP:
                                emit_loads(hp + 2)
                                emit_s_exp(hp + 2)
                            emit_norm(hp, oAB)

                    # ================= Phase 4: out projection + residual =========
                    with tc.tile_pool(name="p4", bufs=1) as p4, \
                         tc.tile_pool(name="p4o", bufs=2) as p4o, \
                         tc.tile_pool(name="p4psum", bufs=2, space="PSUM") as p4psum:
                        wo_sb = p4.tile([P, NDT, D], bf16)
                        nc.sync.dma_start(
                            out=wo_sb, in_=woT.rearrange("(dt p) m -> p dt m", p=P)
                        )
                        for tt in range(NTT):
                            ob = p4o.tile([P, D], f32, tag="ob")
                            for mc in range(2):
                                ps = p4psum.tile([P, TPC], f32, tag="ops")
                                for dt in range(NDT):
                                    nc.tensor.matmul(
                                        ps,
                                        aO_sb[:, dt, tt * P:(tt + 1) * P],
                                        wo_sb[:, dt, mc * 512:(mc + 1) * 512],
                                        start=(dt == 0),
                                        stop=(dt == NDT - 1),
                                    )
                                nc.vector.tensor_add(
                                    ob[:, mc * 512:(mc + 1) * 512],
                                    ps,
                                    q_sb[:, tt, mc * 512:(mc + 1) * 512],
                                )
                            nc.sync.dma_start(
                                out=out_c[tt * P:(tt + 1) * P, :], in_=ob
                            )

    nc.compile()
    return nc


def _get_nc():
    if "nc" not in _CACHE:
        _CACHE["nc"] = build_nc()
    return _CACHE["nc"]


def make_in_maps(q, k, v, w_q, w_k, w_v, w_o, ln_g, ln_b):
    import ml_dtypes

    bf = ml_dtypes.bfloat16
    q2 = np.ascontiguousarray(q.reshape(NT, D), dtype=np.float32)
    kT = np.ascontiguousarray(k.reshape(NT, D).T.astype(bf))
    vT = np.ascontiguousarray(v.reshape(NT, D).T.astype(bf))
    wgqT = np.ascontiguousarray((w_q * ln_g[None, :]).T.astype(bf))
    wkT = np.ascontiguousarray(w_k.T.astype(bf))
    wvT = np.ascontiguousarray(w_v.T.astype(bf))
    woT = np.ascontiguousarray(w_o.T.astype(bf))
    cq = np.ascontiguousarray(w_q @ ln_b, dtype=np.float32)
    in_maps = []
    for c in range(N_CORES):
        sl = slice(c * TPC, (c + 1) * TPC)
        in_maps.append(
            {
                "q_c": q2[sl],
                "kT_c": np.ascontiguousarray(kT[:, sl]),
                "vT_c": np.ascontiguousarray(vT[:, sl]),
                "wgqT": wgqT,
                "wkT": wkT,
                "wvT": wvT,
                "woT": woT,
                "cq": cq,
            }
        )
    return in_maps


def run(inputs, trace=False, tmpdir=None):
    """Run the device kernel.  Returns (out [B, L, D], BassKernelResults)."""
    from concourse.bass_utils import run_bass_kernel_spmd

    nc = _get_nc()
    in_maps = make_in_maps(
        inputs["q"], inputs["k"], inputs["v"], inputs["w_q"], inputs["w_k"],
        inputs["w_v"], inputs["w_o"], inputs["ln_g"], inputs["ln_b"],
    )
    res = run_bass_kernel_spmd(
        nc, in_maps, list(range(N_CORES)), trace=trace, tmpdir=tmpdir
    )
    rows = np.concatenate([res.results[c]["out_c"] for c in range(N_CORES)], axis=0)
    return rows.reshape(B, L, D), res


def kernel(q, k, v, mask, w_q, w_k, w_v, w_o, ln_g, ln_b):
    q = np.asarray(q, dtype=np.float32)
    k = np.asarray(k, dtype=np.float32)
    v = np.asarray(v, dtype=np.float32)
    mask = np.asarray(mask)
    w_q = np.asarray(w_q, dtype=np.float32)
    w_k = np.asarray(w_k, dtype=np.float32)
    w_v = np.asarray(w_v, dtype=np.float32)
    w_o = np.asarray(w_o, dtype=np.float32)
    ln_g = np.asarray(ln_g, dtype=np.float32)
    ln_b = np.asarray(ln_b, dtype=np.float32)
    if not np.all(mask == 1):
        return _np_reference(q, k, v, mask, w_q, w_k, w_v, w_o, ln_g, ln_b)
    out, _ = run(
        {"q": q, "k": k, "v": v, "w_q": w_q, "w_k": w_k, "w_v": w_v,
         "w_o": w_o, "ln_g": ln_g, "ln_b": ln_b},
        trace=False,
    )
    return out
